# revision 5
# baseline (speedup 1.0000x reference)
"""Trainium2 Bass kernel for nn_ContextualRQTransformer.

Sharding: 8 cores = 4 batch elements x 2 sequence halves.
Core c owns batch b=c//2, tokens [hf*512, hf*512+512) with hf=c%2.
Encoder is sequence-parallel with a pairwise AllGather of each layer's
output (full 1024-token context is needed for K/V). The depth decoder
is independent per position given the encoder context: zero comms.

Device layout: activations are feature-major (X^T: [features on
partitions, tokens on free]); every linear layer is lhsT=W^T
(pre-transposed on host), rhs=X^T on the PE, accumulated in fp32 PSUM.
Matmul operands are bf16; the residual/LayerNorm stream is kept in
fp32 ("twin" f32+bf16 tiles) to bound accumulated rounding error.
Attention uses transposed scores S^T [keys, q]; softmax denominators
come from an all-ones column appended to token-major V tiles (V_aug),
so the AV matmul stream produces O^T and the sums together. LayerNorm
reductions (over features = partitions) are all-ones matmuls on the
PE; rstd = exp(-0.5*ln(var+eps)) keeps ACT on the single exp/ln table
set for the whole kernel.
"""

import sys
import numpy as np

sys.path.insert(0, "/opt/trn_rl_repo")

import ml_dtypes

L, D, H, F, V, B, S = 4, 256, 8, 2048, 1024, 4, 1024
HD = D // H          # 32
TOK = 512            # tokens owned per core
NC = 8               # cores
C = D // 128         # feature c-tiles (2)
KT = S // 128        # key tiles (8)
FT = F // 128        # ffn tiles (16)
EPS = 1e-5
ISCALE = 1.0 / np.float32(np.sqrt(HD))
GROUPS = [[0, 1], [2, 3], [4, 5], [6, 7]]
BF = ml_dtypes.bfloat16

_runner = None


def _build():
    from concourse import bass, mybir, tile, bacc

    f32 = mybir.dt.float32
    bf16 = mybir.dt.bfloat16
    AF = mybir.ActivationFunctionType
    ALU = mybir.AluOpType

    nc = bacc.Bacc(num_devices=NC)

    def din(name, shape, dt=bf16):
        return nc.declare_dram_parameter(name, list(shape), dt, isOutput=False)

    h_fullT = din("h_fullT", [D, S])
    h_ownTf = din("h_ownTf", [D, TOK], f32)
    enc_qkvT = din("enc_qkvT", [L, D, 3 * D])
    enc_outT = din("enc_outT", [L, D, D])
    enc_ff1T = din("enc_ff1T", [L, D, F])
    enc_ff2T = din("enc_ff2T", [L, F, D])
    dec_sa_vT = din("dec_sa_vT", [L, D, D])
    dec_sa_outT = din("dec_sa_outT", [L, D, D])
    dec_ca_qkvT = din("dec_ca_qkvT", [L, D, 3 * D])
    dec_ca_outT = din("dec_ca_outT", [L, D, D])
    dec_ff1T = din("dec_ff1T", [L, D, F])
    dec_ff2T = din("dec_ff2T", [L, F, D])
    enc_qkv_bc = din("enc_qkv_bc", [L, 128, 6], f32)
    enc_out_bc = din("enc_out_bc", [L, 128, 2], f32)
    enc_ff1_bc = din("enc_ff1_bc", [L, 128, 16], f32)
    enc_ff2_bc = din("enc_ff2_bc", [L, 128, 2], f32)
    enc_bv = din("enc_bv", [L, D], f32)
    enc_ln1_gc = din("enc_ln1_gc", [L, 128, 2], f32)
    enc_ln1_bc = din("enc_ln1_bc", [L, 128, 2], f32)
    enc_ln2_gc = din("enc_ln2_gc", [L, 128, 2], f32)
    enc_ln2_bc = din("enc_ln2_bc", [L, 128, 2], f32)
    dec_sa_v_bc = din("dec_sa_v_bc", [L, 128, 2], f32)
    dec_sa_out_bc = din("dec_sa_out_bc", [L, 128, 2], f32)
    dec_ca_qkv_bc = din("dec_ca_qkv_bc", [L, 128, 6], f32)
    dec_ca_out_bc = din("dec_ca_out_bc", [L, 128, 2], f32)
    dec_ca_bv = din("dec_ca_bv", [L, D], f32)
    dec_ff1_bc = din("dec_ff1_bc", [L, 128, 16], f32)
    dec_ff2_bc = din("dec_ff2_bc", [L, 128, 2], f32)
    dec_ln1_gc = din("dec_ln1_gc", [L, 128, 2], f32)
    dec_ln1_bc = din("dec_ln1_bc", [L, 128, 2], f32)
    dec_ln2_gc = din("dec_ln2_gc", [L, 128, 2], f32)
    dec_ln2_bc = din("dec_ln2_bc", [L, 128, 2], f32)
    dec_ln3_gc = din("dec_ln3_gc", [L, 128, 2], f32)
    dec_ln3_bc = din("dec_ln3_bc", [L, 128, 2], f32)

    yout = nc.declare_dram_parameter("y", [D, TOK], f32, isOutput=True)

    with tile.TileContext(nc) as tc:
      with (
        nc.allow_low_precision(reason="bf16 matmul transformer kernel"),
        tc.tile_pool(name="const", bufs=1) as constp,
        tc.tile_pool(name="wts", bufs=2) as wts,
        tc.tile_pool(name="acts", bufs=2) as acts,
        tc.tile_pool(name="attn", bufs=2) as attnp,
        tc.tile_pool(name="psum", bufs=2, space="PSUM") as psum,
        tc.tile_pool(name="dram", bufs=2, space="DRAM") as dram,
      ):
        ones128 = constp.tile([128, 128], bf16)
        nc.vector.memset(ones128[:], 1.0)
        epst = constp.tile([128, 1], f32)
        nc.vector.memset(epst[:], EPS)

        def load_w(src_ap, shape, tag, bufs=2):
            t = wts.tile(shape, src_ap.dtype, tag=tag, bufs=bufs, name=tag)
            nc.sync.dma_start(out=t[:], in_=src_ap)
            return t

        def mm_fm(rhs_tiles, wT, bias_col, out_tag, resid=None, twins=False,
                  col0=0):
            """Feature-major linear: out^T[m] = sum_c wT[c][:,...]^T@rhs[c].

            resid: list of f32 tiles added in. twins: also emit f32 copy.
            Returns outs_b or (outs_f, outs_b).
            """
            outs_f, outs_b = [], []
            for m in range(C):
                p = psum.tile([128, TOK], f32, tag="mmp", bufs=2, name="mmp")
                for c in range(C):
                    nc.tensor.matmul(
                        p[:],
                        wT[c][:, col0 + m * 128:col0 + (m + 1) * 128],
                        rhs_tiles[c][:],
                        start=(c == 0), stop=(c == C - 1))
                bcol = bias_col[:, m:m + 1]
                if resid is not None:
                    if twins:
                        of = acts.tile([128, TOK], f32, tag="resf",
                                       bufs=4, name=out_tag + "f")
                        nc.vector.scalar_tensor_tensor(
                            out=of[:], in0=p[:], scalar=bcol,
                            in1=resid[m][:], op0=ALU.add, op1=ALU.add)
                        outs_f.append(of)
                        ob = acts.tile([128, TOK], bf16, tag="resb",
                                       bufs=4, name=out_tag + "b")
                        nc.vector.tensor_copy(out=ob[:], in_=of[:])
                        outs_b.append(ob)
                    else:
                        ob = acts.tile([128, TOK], bf16, tag=out_tag, bufs=2,
                                       name=out_tag)
                        nc.vector.scalar_tensor_tensor(
                            out=ob[:], in0=p[:], scalar=bcol,
                            in1=resid[m][:], op0=ALU.add, op1=ALU.add)
                        outs_b.append(ob)
                else:
                    ob = acts.tile([128, TOK], bf16, tag="qsb", bufs=4,
                                   name=out_tag)
                    nc.vector.tensor_scalar(
                        out=ob[:], in0=p[:], scalar1=bcol, scalar2=None,
                        op0=ALU.add)
                    outs_b.append(ob)
            if twins:
                return outs_f, outs_b
            return outs_b

        def layernorm(yf, yb, g_col, b_col, out_tag, want_f32=True):
            """LN over features. Stats from bf16 twins (matmul-reducible),
            applied to the f32 stream. Returns (outs_f|None, outs_b)."""
            sump = psum.tile([128, TOK], f32, tag="mmp", bufs=2, name="lnsum")
            for c in range(C):
                nc.tensor.matmul(sump[:], ones128[:], yb[c][:],
                                 start=(c == 0), stop=(c == C - 1))
            sqs = []
            for c in range(C):
                sq = acts.tile([128, TOK], bf16, tag="lnsq", bufs=2,
                               name="lnsq")
                nc.vector.tensor_tensor(out=sq[:], in0=yb[c][:], in1=yb[c][:],
                                        op=ALU.mult)
                sqs.append(sq)
            sqp = psum.tile([128, TOK], f32, tag="mmp", bufs=2, name="lnsqp")
            for c in range(C):
                nc.tensor.matmul(sqp[:], ones128[:], sqs[c][:],
                                 start=(c == 0), stop=(c == C - 1))
            m_sb = acts.tile([128, TOK], f32, tag="lnA", bufs=2, name="lnm")
            nc.vector.tensor_scalar(out=m_sb[:], in0=sump[:], scalar1=1.0 / D,
                                    scalar2=None, op0=ALU.mult)
            rstd = acts.tile([128, TOK], f32, tag="lnB", bufs=2, name="lnB")
            nc.vector.tensor_tensor(out=rstd[:], in0=m_sb[:], in1=m_sb[:],
                                    op=ALU.mult)
            nc.vector.scalar_tensor_tensor(
                out=rstd[:], in0=sqp[:], scalar=1.0 / D, in1=rstd[:],
                op0=ALU.mult, op1=ALU.subtract)
            nc.scalar.activation(out=rstd[:], in_=rstd[:], func=AF.Ln,
                                 bias=epst[:], scale=1.0)
            nc.scalar.activation(out=rstd[:], in_=rstd[:], func=AF.Exp,
                                 bias=0.0, scale=-0.5)
            outs_f, outs_b = [], []
            for c in range(C):
                t2 = acts.tile([128, TOK], f32, tag="lnC", bufs=3, name="lnC")
                nc.vector.tensor_tensor(out=t2[:], in0=yf[c][:], in1=m_sb[:],
                                        op=ALU.subtract)
                nc.vector.tensor_tensor(out=t2[:], in0=t2[:], in1=rstd[:],
                                        op=ALU.mult)
                ob = acts.tile([128, TOK], bf16, tag="lnob", bufs=6,
                               name=out_tag + "b")
                nc.vector.tensor_scalar(
                    out=ob[:], in0=t2[:], scalar1=g_col[:, c:c + 1],
                    scalar2=b_col[:, c:c + 1], op0=ALU.mult, op1=ALU.add)
                outs_b.append(ob)
                if want_f32:
                    of = acts.tile([128, TOK], f32, tag="lnof", bufs=6,
                                   name=out_tag + "f")
                    nc.vector.tensor_scalar(
                        out=of[:], in0=t2[:], scalar1=g_col[:, c:c + 1],
                        scalar2=b_col[:, c:c + 1], op0=ALU.mult, op1=ALU.add)
                    outs_f.append(of)
            return (outs_f if want_f32 else None), outs_b

        def attention(q_src, ctxT, qkvT_t, qkv_bc_t, bv_ap, outT_t, out_bc_t,
                      resid, out_tag):
            """Full MHA: Q from q_src (own tokens, bf16), K/V from ctxT."""
            q_sb = mm_fm(q_src, qkvT_t, qkv_bc_t, "q_sb")
            k_sb = []
            for m in range(C):
                kt_tile = acts.tile([128, S], bf16, tag="k_sb", bufs=2,
                                    name="k_sb")
                for nh in range(2):
                    p = psum.tile([128, 512], f32, tag="mmp", bufs=2,
                                  name="kp")
                    for c in range(C):
                        nc.tensor.matmul(
                            p[:],
                            qkvT_t[c][:, D + m * 128:D + (m + 1) * 128],
                            ctxT[c][:, nh * 512:(nh + 1) * 512],
                            start=(c == 0), stop=(c == C - 1))
                    nc.vector.tensor_scalar(
                        out=kt_tile[:, nh * 512:(nh + 1) * 512], in0=p[:],
                        scalar1=qkv_bc_t[:, 2 + m:3 + m], scalar2=None,
                        op0=ALU.add)
                k_sb.append(kt_tile)
            bv_rep = wts.tile([128, H, HD], f32, tag="bv_rep", bufs=2,
                              name="bv_rep")
            bvr = bv_ap.rearrange("(h j) -> h j", h=H)
            nc.sync.dma_start(
                out=bv_rep[:],
                in_=bass.AP(tensor=bvr.tensor, offset=bvr.offset,
                            ap=[[0, 128]] + list(bvr.ap)))
            vaug = []
            for kt in range(KT):
                p = psum.tile([128, D], f32, tag="mmp", bufs=2, name="vp")
                for c in range(C):
                    nc.tensor.matmul(
                        p[:], ctxT[c][:, kt * 128:(kt + 1) * 128],
                        qkvT_t[c][:, 2 * D:3 * D],
                        start=(c == 0), stop=(c == C - 1))
                va = attnp.tile([128, H, HD + 1], bf16, tag="vaug", bufs=10,
                                name="vaug")
                nc.vector.memset(va[:, :, HD:HD + 1], 1.0)
                nc.vector.tensor_tensor(
                    out=va[:, :, 0:HD],
                    in0=p.rearrange("p (h j) -> p h j", h=H),
                    in1=bv_rep[:], op=ALU.add)
                vaug.append(va)
            opack = [attnp.tile([128, TOK], bf16, tag="opack", bufs=2,
                                name="opack") for _ in range(C)]
            for h in range(H):
                qc, qs = h // 4, (h % 4) * 32
                avp = psum.tile([HD + 1, TOK], f32, tag="avp", bufs=2,
                                name="avp")
                for kt in range(KT):
                    sp = psum.tile([128, TOK], f32, tag="scorep", bufs=2,
                                   name="scorep")
                    nc.tensor.matmul(
                        sp[:],
                        k_sb[qc][qs:qs + 32, kt * 128:(kt + 1) * 128],
                        q_sb[qc][qs:qs + 32, :],
                        start=True, stop=True, tile_position=(qs, 0))
                    es = attnp.tile([128, TOK], bf16, tag="expS", bufs=3,
                                    name="expS")
                    nc.scalar.activation(out=es[:], in_=sp[:], func=AF.Exp,
                                         bias=0.0, scale=float(ISCALE))
                    nc.tensor.matmul(avp[:], vaug[kt][:, h, :], es[:],
                                     start=(kt == 0), stop=(kt == KT - 1))
                rrow = attnp.tile([HD + 1, TOK], bf16, tag="rrow", bufs=2,
                                  name="rrow")
                nc.vector.reciprocal(out=rrow[HD:HD + 1, :],
                                     in_=avp[HD:HD + 1, :])
                rp = psum.tile([HD, TOK], f32, tag="scorep", bufs=2, name="rp")
                nc.tensor.matmul(rp[:], ones128[HD:HD + 1, 0:HD],
                                 rrow[HD:HD + 1, :], start=True, stop=True,
                                 tile_position=(HD, 0))
                rr_sb = attnp.tile([HD, TOK], f32, tag="rr_sb", bufs=2,
                                   name="rr_sb")
                nc.vector.tensor_copy(out=rr_sb[:], in_=rp[:])
                oh = attnp.tile([HD, TOK], bf16, tag="oh", bufs=3, name="oh")
                nc.vector.tensor_tensor(out=oh[:], in0=avp[0:HD, :],
                                        in1=rr_sb[:], op=ALU.mult)
                nc.sync.dma_start(out=opack[qc][qs:qs + 32, :], in_=oh[:])
            return mm_fm(opack, outT_t, out_bc_t, out_tag, resid=resid,
                         twins=True)

        def ffn(x2b, ff1T_t, ff1_bc_t, ff2T_t, ff2_bc_t, resid, out_tag):
            accs = [psum.tile([128, TOK], f32, tag="acc", bufs=2, name="acc")
                    for _ in range(C)]
            for ft in range(FT):
                p = psum.tile([128, TOK], f32, tag="mmp", bufs=2, name="ffp")
                for c in range(C):
                    nc.tensor.matmul(p[:],
                                     ff1T_t[c][:, ft * 128:(ft + 1) * 128],
                                     x2b[c][:], start=(c == 0),
                                     stop=(c == C - 1))
                r = acts.tile([128, TOK], bf16, tag="relu", bufs=3, name="relu")
                if ft % 2 == 0:
                    nc.scalar.activation(out=r[:], in_=p[:], func=AF.Relu,
                                         bias=ff1_bc_t[:, ft:ft + 1],
                                         scale=1.0)
                else:
                    nc.vector.tensor_scalar(
                        out=r[:], in0=p[:], scalar1=ff1_bc_t[:, ft:ft + 1],
                        scalar2=0.0, op0=ALU.add, op1=ALU.max)
                for m in range(C):
                    nc.tensor.matmul(accs[m][:],
                                     ff2T_t[ft][:, m * 128:(m + 1) * 128],
                                     r[:], start=(ft == 0),
                                     stop=(ft == FT - 1))
            outs_f, outs_b = [], []
            for m in range(C):
                of = acts.tile([128, TOK], f32, tag="resf", bufs=4,
                               name=out_tag + "f")
                nc.vector.scalar_tensor_tensor(
                    out=of[:], in0=accs[m][:], scalar=ff2_bc_t[:, m:m + 1],
                    in1=resid[m][:], op0=ALU.add, op1=ALU.add)
                outs_f.append(of)
                ob = acts.tile([128, TOK], bf16, tag="resb", bufs=4,
                               name=out_tag + "b")
                nc.vector.tensor_copy(out=ob[:], in_=of[:])
                outs_b.append(ob)
            return outs_f, outs_b

        # ---- initial loads ----
        ctxT = []
        for c in range(C):
            t = acts.tile([128, S], bf16, tag="ctx", bufs=2, name="ctx")
            nc.sync.dma_start(out=t[:], in_=h_fullT[c * 128:(c + 1) * 128, :])
            ctxT.append(t)
        xownf, xownb = [], []
        for c in range(C):
            tf = acts.tile([128, TOK], f32, tag="lnof", bufs=6, name="xownf")
            nc.sync.dma_start(out=tf[:], in_=h_ownTf[c * 128:(c + 1) * 128, :])
            xownf.append(tf)
            tb = acts.tile([128, TOK], bf16, tag="lnob", bufs=6, name="xownb")
            nc.vector.tensor_copy(out=tb[:], in_=tf[:])
            xownb.append(tb)

        # ---- encoder ----
        for l in range(L):
            qkvT_t = [load_w(enc_qkvT[l, c * 128:(c + 1) * 128, :],
                             [128, 3 * D], f"qkv{c}") for c in range(C)]
            outT_t = [load_w(enc_outT[l, c * 128:(c + 1) * 128, :],
                             [128, D], f"ow{c}", bufs=6) for c in range(C)]
            qkv_bc_t = load_w(enc_qkv_bc[l], [128, 6], "bias", bufs=24)
            out_bc_t = load_w(enc_out_bc[l], [128, 2], "bias", bufs=24)
            ln1g = load_w(enc_ln1_gc[l], [128, 2], "bias", bufs=24)
            ln1b = load_w(enc_ln1_bc[l], [128, 2], "bias", bufs=24)
            ln2g = load_w(enc_ln2_gc[l], [128, 2], "bias", bufs=24)
            ln2b = load_w(enc_ln2_bc[l], [128, 2], "bias", bufs=24)

            y1f, y1b = attention(xownb, ctxT, qkvT_t, qkv_bc_t, enc_bv[l],
                                 outT_t, out_bc_t, xownf, "ey1")
            x2f, x2b = layernorm(y1f, y1b, ln1g, ln1b, "ex2")

            ff1T_t = [load_w(enc_ff1T[l, c * 128:(c + 1) * 128, :],
                             [128, F], f"ff1{c}") for c in range(C)]
            ff2T_t = [load_w(enc_ff2T[l, ft * 128:(ft + 1) * 128, :],
                             [128, D], f"ff2{ft}") for ft in range(FT)]
            ff1_bc_t = load_w(enc_ff1_bc[l], [128, 16], "bias", bufs=24)
            ff2_bc_t = load_w(enc_ff2_bc[l], [128, 2], "bias", bufs=24)
            y2f, y2b = ffn(x2b, ff1T_t, ff1_bc_t, ff2T_t, ff2_bc_t, x2f, "ey2")
            xownf, xownb = layernorm(y2f, y2b, ln2g, ln2b, "exo")

            ag_in = dram.tile([D, TOK], bf16, name="ag_in")
            for c in range(C):
                nc.sync.dma_start(out=ag_in[c * 128:(c + 1) * 128, :],
                                  in_=xownb[c][:])
            ag_out = dram.tile([2, D, TOK], bf16, name="ag_out")
            nc.gpsimd.collective_compute(
                "AllGather", mybir.AluOpType.bypass, replica_groups=GROUPS,
                ins=[ag_in.opt()], outs=[ag_out.opt()])
            ctxT = []
            for c in range(C):
                t = acts.tile([128, S], bf16, tag="ctx", bufs=2, name="ctx")
                nc.sync.dma_start(
                    out=t.rearrange("p (t n) -> p t n", t=2),
                    in_=ag_out.rearrange("t p n -> p t n")[
                        c * 128:(c + 1) * 128])
                ctxT.append(t)

        # ---- decoder ----
        downf, downb = [], []
        for c in range(C):
            tf = acts.tile([128, TOK], f32, tag="lnof", bufs=6, name="downf")
            nc.sync.dma_start(out=tf[:], in_=h_ownTf[c * 128:(c + 1) * 128, :])
            downf.append(tf)
            tb = acts.tile([128, TOK], bf16, tag="lnob", bufs=6, name="downb")
            nc.vector.tensor_copy(out=tb[:], in_=tf[:])
            downb.append(tb)

        for l in range(L):
            sa_vT_t = [load_w(dec_sa_vT[l, c * 128:(c + 1) * 128, :],
                              [128, D], f"ow{c}", bufs=6) for c in range(C)]
            sa_oT_t = [load_w(dec_sa_outT[l, c * 128:(c + 1) * 128, :],
                              [128, D], f"ow{c}", bufs=6) for c in range(C)]
            sa_v_bc_t = load_w(dec_sa_v_bc[l], [128, 2], "bias", bufs=24)
            sa_o_bc_t = load_w(dec_sa_out_bc[l], [128, 2], "bias", bufs=24)
            ln1g = load_w(dec_ln1_gc[l], [128, 2], "bias", bufs=24)
            ln1b = load_w(dec_ln1_bc[l], [128, 2], "bias", bufs=24)

            vsa = mm_fm(downb, sa_vT_t, sa_v_bc_t, "vsa")
            dy1f, dy1b = mm_fm(vsa, sa_oT_t, sa_o_bc_t, "dy1", resid=downf,
                               twins=True)
            d2f, d2b = layernorm(dy1f, dy1b, ln1g, ln1b, "d2")

            ca_qkvT_t = [load_w(dec_ca_qkvT[l, c * 128:(c + 1) * 128, :],
                                [128, 3 * D], f"qkv{c}") for c in range(C)]
            ca_outT_t = [load_w(dec_ca_outT[l, c * 128:(c + 1) * 128, :],
                                [128, D], f"ow{c}", bufs=6) for c in range(C)]
            ca_qkv_bc_t = load_w(dec_ca_qkv_bc[l], [128, 6], "bias", bufs=24)
            ca_out_bc_t = load_w(dec_ca_out_bc[l], [128, 2], "bias", bufs=24)
            ln2g = load_w(dec_ln2_gc[l], [128, 2], "bias", bufs=24)
            ln2b = load_w(dec_ln2_bc[l], [128, 2], "bias", bufs=24)

            dy2f, dy2b = attention(d2b, ctxT, ca_qkvT_t, ca_qkv_bc_t,
                                   dec_ca_bv[l], ca_outT_t, ca_out_bc_t,
                                   d2f, "dy2")
            d3f, d3b = layernorm(dy2f, dy2b, ln2g, ln2b, "d3")

            ff1T_t = [load_w(dec_ff1T[l, c * 128:(c + 1) * 128, :],
                             [128, F], f"ff1{c}") for c in range(C)]
            ff2T_t = [load_w(dec_ff2T[l, ft * 128:(ft + 1) * 128, :],
                             [128, D], f"ff2{ft}") for ft in range(FT)]
            ff1_bc_t = load_w(dec_ff1_bc[l], [128, 16], "bias", bufs=24)
            ff2_bc_t = load_w(dec_ff2_bc[l], [128, 2], "bias", bufs=24)
            ln3g = load_w(dec_ln3_gc[l], [128, 2], "bias", bufs=24)
            ln3b = load_w(dec_ln3_bc[l], [128, 2], "bias", bufs=24)
            dy3f, dy3b = ffn(d3b, ff1T_t, ff1_bc_t, ff2T_t, ff2_bc_t, d3f,
                             "dy3")
            last = (l == L - 1)
            downf, downb = layernorm(dy3f, dy3b, ln3g, ln3b, "dwn",
                                     want_f32=True)

        for c in range(C):
            nc.sync.dma_start(out=yout[c * 128:(c + 1) * 128, :],
                              in_=downf[c][:])

    nc.compile()
    return nc


class _Runner:
    """Caches the compiled NEFF + jitted shard_map across kernel() calls."""

    def __init__(self):
        import jax
        from jax.experimental.shard_map import shard_map
        from jax.sharding import Mesh, PartitionSpec
        from concourse import bass2jax, mybir

        self.jax = jax
        nc = _build()
        self.nc = nc
        bass2jax.install_neuronx_cc_hook()

        partition_name0 = (nc.partition_id_tensor.name
                           if nc.partition_id_tensor else None)
        in_names, out_names, out_avals, zero_shapes = [], [], [], []
        for alloc in nc.m.functions[0].allocations:
            if not isinstance(alloc, mybir.MemoryLocationSet):
                continue
            name = alloc.memorylocations[0].name
            if alloc.kind == "ExternalInput":
                if name != partition_name0:
                    in_names.append(name)
            elif alloc.kind == "ExternalOutput":
                out_names.append(name)
                shape = tuple(alloc.tensor_shape)
                dtype = mybir.dt.np(alloc.dtype)
                out_avals.append(jax.core.ShapedArray(shape, dtype))
                zero_shapes.append((shape, dtype))
        self.n_params = len(in_names)
        self.in_names = list(in_names)
        self.out_names = out_names
        self.out_avals = out_avals
        self.zero_shapes = zero_shapes

        all_names = list(in_names) + list(out_names)
        partition_name = (nc.partition_id_tensor.name
                          if nc.partition_id_tensor else None)
        if partition_name is not None:
            all_names.append(partition_name)

        def _body(*args):
            operands = list(args)
            if partition_name is not None:
                operands.append(bass2jax.partition_id_tensor())
            outs = bass2jax._bass_exec_p.bind(
                *operands,
                out_avals=tuple(out_avals),
                in_names=tuple(all_names),
                out_names=tuple(out_names),
                lowering_input_output_aliases=(),
                sim_require_finite=True,
                sim_require_nnan=True,
                nc=nc,
            )
            return tuple(outs)

        devices = jax.devices()[:NC]
        self.mesh = Mesh(np.asarray(devices), ("core",))
        n_out = len(out_names)
        in_specs = (PartitionSpec("core"),) * (self.n_params + n_out)
        out_specs = (PartitionSpec("core"),) * n_out
        donate = tuple(range(self.n_params, self.n_params + n_out))
        self.sharded = jax.jit(
            shard_map(_body, mesh=self.mesh, in_specs=in_specs,
                      out_specs=out_specs, check_rep=False),
            donate_argnums=donate, keep_unused=True)
        self._dev_in = None

    def stage(self, in_maps):
        concat = [np.concatenate([np.asarray(in_maps[c][n])
                                  for c in range(NC)], axis=0)
                  for n in self.in_names]
        self._dev_in = [self.jax.device_put(a) for a in concat]
        for a in self._dev_in:
            a.block_until_ready()

    def _zeros(self):
        return [np.zeros((NC * s[0], *s[1:]), dt)
                for s, dt in self.zero_shapes]

    def run(self, n_iters=1):
        outs = None
        for _ in range(n_iters):
            outs = self.sharded(*self._dev_in, *self._zeros())
        for o in outs:
            o.block_until_ready()
        return outs

    def results(self, outs):
        res = []
        for c in range(NC):
            res.append({
                name: np.asarray(outs[i]).reshape(
                    NC, *self.out_avals[i].shape)[c]
                for i, name in enumerate(self.out_names)})
        return res


def _col_layout(b):
    n = b.shape[-1]
    return np.ascontiguousarray(
        b.reshape(*b.shape[:-1], n // 128, 128).swapaxes(-1, -2)
    ).astype(np.float32)


def _prepare_in_maps(inputs):
    ii = {k: np.asarray(v) for k, v in inputs.items()}
    x = ii["x"].astype(np.int64)
    codebook = ii["codebook"].astype(np.float32)
    pos = ii["pos"].astype(np.float32)
    h = codebook[x] + pos[0, :S][None, :, :]        # [B, S, D] fp32
    # mask is all-False by construction (spec fill=zeros) -> no-op

    wmap = {
        "enc_qkvT": ii["enc_qkv_w"].transpose(0, 2, 1),
        "enc_outT": ii["enc_out_w"].transpose(0, 2, 1),
        "enc_ff1T": ii["enc_ff1_w"].transpose(0, 2, 1),
        "enc_ff2T": ii["enc_ff2_w"].transpose(0, 2, 1),
        "dec_sa_vT": ii["dec_sa_qkv_w"][:, 2 * D:, :].transpose(0, 2, 1),
        "dec_sa_outT": ii["dec_sa_out_w"].transpose(0, 2, 1),
        "dec_ca_qkvT": ii["dec_ca_qkv_w"].transpose(0, 2, 1),
        "dec_ca_outT": ii["dec_ca_out_w"].transpose(0, 2, 1),
        "dec_ff1T": ii["dec_ff1_w"].transpose(0, 2, 1),
        "dec_ff2T": ii["dec_ff2_w"].transpose(0, 2, 1),
    }
    wmap = {k: np.ascontiguousarray(v).astype(BF) for k, v in wmap.items()}
    bmap = {
        "enc_qkv_bc": _col_layout(ii["enc_qkv_b"]),
        "enc_out_bc": _col_layout(ii["enc_out_b"]),
        "enc_ff1_bc": _col_layout(ii["enc_ff1_b"]),
        "enc_ff2_bc": _col_layout(ii["enc_ff2_b"]),
        "enc_bv": ii["enc_qkv_b"][:, 2 * D:].astype(np.float32),
        "enc_ln1_gc": _col_layout(ii["enc_ln1_g"]),
        "enc_ln1_bc": _col_layout(ii["enc_ln1_b"]),
        "enc_ln2_gc": _col_layout(ii["enc_ln2_g"]),
        "enc_ln2_bc": _col_layout(ii["enc_ln2_b"]),
        "dec_sa_v_bc": _col_layout(ii["dec_sa_qkv_b"][:, 2 * D:]),
        "dec_sa_out_bc": _col_layout(ii["dec_sa_out_b"]),
        "dec_ca_qkv_bc": _col_layout(ii["dec_ca_qkv_b"]),
        "dec_ca_out_bc": _col_layout(ii["dec_ca_out_b"]),
        "dec_ca_bv": ii["dec_ca_qkv_b"][:, 2 * D:].astype(np.float32),
        "dec_ff1_bc": _col_layout(ii["dec_ff1_b"]),
        "dec_ff2_bc": _col_layout(ii["dec_ff2_b"]),
        "dec_ln1_gc": _col_layout(ii["dec_ln1_g"]),
        "dec_ln1_bc": _col_layout(ii["dec_ln1_b"]),
        "dec_ln2_gc": _col_layout(ii["dec_ln2_g"]),
        "dec_ln2_bc": _col_layout(ii["dec_ln2_b"]),
        "dec_ln3_gc": _col_layout(ii["dec_ln3_g"]),
        "dec_ln3_bc": _col_layout(ii["dec_ln3_b"]),
    }
    in_maps = []
    for c in range(NC):
        b, hf = c // 2, c % 2
        hT = np.ascontiguousarray(h[b].T)
        m = {"h_fullT": hT.astype(BF),
             "h_ownTf": np.ascontiguousarray(
                 hT[:, hf * TOK:(hf + 1) * TOK]).astype(np.float32)}
        m.update(wmap)
        m.update(bmap)
        in_maps.append(m)
    return in_maps


def get_runner():
    global _runner
    if _runner is None:
        _runner = _Runner()
    return _runner


def kernel(**inputs):
    r = get_runner()
    r.stage(_prepare_in_maps(inputs))
    res = r.results(r.run(1))
    out = np.empty((B, S, D), np.float32)
    for c in range(NC):
        b, hf = c // 2, c % 2
        out[b, hf * TOK:(hf + 1) * TOK, :] = res[c]["y"].T
    return out
